# revision 18
# baseline (speedup 1.0000x reference)
"""BatchHardTripletLoss on 8 Trainium2 NeuronCores — fp8 version.

Strategy (data-parallel over anchor rows, samples pre-sorted by label):
  - host sorts samples by label; core c owns anchor rows [c*512, (c+1)*512).
  - mining domain is flipped so the embedding matrix is used unsigned on
    both matmul sides (no separate negated copy):
        w = 2 e_m.e_n - colterm_n - PEN*same(m,n)
    where colterm_n = |e_n|^2 - 2*eps*sum(e_n).  Then per anchor row m:
        row max of w  -> hn2 = rowterm_m - max   (hardest negative^2)
        row min of w  -> hp2 = rowterm_m - PEN - min  (hardest positive^2)
    PEN pushes same-label columns below every non-same value, so the
    global max is the hardest negative and the global min lands on the
    hardest positive (self-column has d2=0 and never wins generically;
    rows without positives are masked on the host via label counts).
  - gram matmuls run in fp8e4 DoubleRow (2 cols/cycle), contracting 256
    dims per instruction; embeddings are scaled by sqrt(2) so the gram
    directly yields 2 e.e.  colterm/PEN are injected by one K=128 fp8
    matmul per psum slice (class-indicator rows + colterm/4 rows).
  - per-core column permutation [own 512 | rest of own group | other
    group] makes the one SPMD program core-agnostic: the stationary
    (lhsT) slices always sit at columns 0:512 of the first group.
  - DVE mines row max+min in one custom pass per [128, 2048] psum tile.
  - host: validity via label bincount, sqrt, mean.
"""

import dataclasses

import numpy as np
import ml_dtypes

import concourse.bacc as bacc
import concourse.mybir as mybir
from concourse.bass_utils import run_bass_kernel_spmd
from concourse.tile import TileContext
from concourse import dve_ops as _dve_ops
from concourse.dve_spec import (
    AluOp, C0, C1, C2, Idx, Spec, Src0, lower, ne, scan, select,
)
from concourse.dve_uop import AluInp, DveOpSpec, InpSel


def _register_dual_op():
    """One DVE pass over a [P, N] tile producing BOTH reductions:
    accum_out = max(seed_s1, in0) over ALL elements,
    out[:, N-1] = running min (seeded +FLT_MAX via imm2) = total row min.

    lower() produces an accum that reduces the *body* (which at index s0
    carries the scan value instead of the element), requiring a one-column
    fixup that reads PSUM and stalls buffer release.  We patch the steady
    uop's accum stage to read the Src0 delay lane directly, so the max
    covers every element and no fixup is needed.
    """
    name = "ANT_MINMAX_DUAL2"
    for op in _dve_ops.OPS:
        if op.name == name:
            return op
    spec = Spec(
        body=select(ne(Idx, C0), Src0, scan(AluOp.MIN, Src0, init=C2)),
        accum=AluOp.MAX,
        accum_init=C1,
        reference=lambda in0, s0, s1, imm2: np.where(
            np.arange(in0.shape[-1]) != s0,
            in0,
            np.minimum.accumulate(np.minimum(in0, imm2), axis=-1),
        ),
    )
    op = _dve_ops.DveOp(name, spec, subdim=False, uops_sha={})
    _dve_ops.OPS.append(op)
    opcode = _dve_ops._CUSTOM_DVE_ROW_BASE + len(_dve_ops.OPS) - 1
    assert opcode < 0x20
    _dve_ops._SUB_OPCODE_FOR_NAME[name] = opcode
    _dve_ops.CUSTOM_DVE_SPECS[name] = spec
    shas = {}
    for ver in ("v3", "v4"):
        uops = lower(spec, ver=ver)
        steady = uops[-1]
        src0_lane = next(
            i - 1 for i, (sel, en) in enumerate(zip(steady.inp,
                                                    steady.inp_enable))
            if en and sel == InpSel.SRC_0)
        patched = False
        for dp in steady.datapath_config:
            if (dp.op == AluOp.MAX and dp.alu_out_a_enable
                    and dp.alu_src0 == AluInp.CURR_ALU_OUT):
                dp.alu_src1 = AluInp(int(AluInp.PREV_DELAY_0) + src0_lane)
                patched = True
                break
        assert patched, f"accum stage not found in {ver} steady uop"
        s = DveOpSpec(name=name, opcode=opcode, uops=uops, rd1_en=False)
        shas[ver] = s.sha(ver)
        _dve_ops._COMPILE_CACHE[(name, ver)] = s
    op = dataclasses.replace(op, uops_sha=shas)
    _dve_ops.OPS[-1] = op
    return op


DUAL_OP = _register_dual_op()

B = 4096          # batch (anchors)
D = 512           # embedding dim
N_CORES = 8
ROWS = B // N_CORES      # 512 anchor rows per core
P = 128                  # partitions
MT = ROWS // P           # 4 m-tiles per core
NW = 512                 # psum bank width (fp32)
GW = 2048                # column group width (4 banks)
NG = B // GW             # 2 column groups
KT = D // P              # 4 contraction tiles
NDR = KT // 2            # 2 DoubleRow matmuls cover K=512

PEN = 16384.0            # same-label penalty; must exceed max d2 (~2.7k here)
KP = 80                  # penalty-matmul contraction (class dims + colterms)
KP_BIG = 128             # fallback when a core has > KP-3 distinct classes
MARGIN = 0.5
EPS = 1e-6

_nc_cache = {}


def _build(reps=1, kp=KP):
    nc = bacc.Bacc("TRN2", target_bir_lowering=False)
    fp8 = mybir.dt.float8e4
    fp16 = mybir.dt.float16
    f32 = mybir.dt.float32

    # [P, KT*GW] partition-major so DMA descriptors are 4KB contiguous runs
    eta = nc.dram_tensor("eta", [P, KT * GW], fp8, kind="ExternalInput")
    etb = nc.dram_tensor("etb", [P, KT * GW], fp8, kind="ExternalInput")
    penl = nc.dram_tensor("penl", [kp, ROWS], fp8, kind="ExternalInput")
    penr = nc.dram_tensor("penr", [kp, B], fp8, kind="ExternalInput")
    outd = nc.dram_tensor("out", [reps, 2 * MT, P], f32, kind="ExternalOutput")

    NEG_INIT = -3.0e38
    SCAN_INIT = 3.0e38
    with TileContext(nc) as tc:
        with (
            tc.tile_pool(name="etp", bufs=1) as etp,
            tc.tile_pool(name="wp", bufs=1) as wp,
            tc.tile_pool(name="accp", bufs=MT) as accp,
            tc.tile_pool(name="psp", bufs=2, space="PSUM") as psp,
        ):
            # --- input DMAs first, split by (group, k-pair) chunks that are
            # contiguous 4KB-per-partition runs on BOTH sides; first chunk on
            # the sync queue so both DGEs issue in parallel.
            et_all = etp.tile([P, NG * KT * GW], fp8, tag="et", name="et_all")
            et6 = et_all.rearrange("p (g k n) -> p g k n", g=NG, k=KT)
            eta2 = eta.rearrange("p (c n) -> p c n", c=2)   # k-pair chunks
            etb2 = etb.rearrange("p (c n) -> p c n", c=2)
            penl_sb = etp.tile([kp, ROWS], fp8, tag="penl")
            penr_sb = etp.tile([kp, B], fp8, tag="penr")
            nc.sync.dma_start(out=et6[:, 0, 0:2, :], in_=eta2[:, 0, :])
            nc.gpsimd.dma_start(out=penl_sb, in_=penl[:, :])
            nc.gpsimd.dma_start(out=et6[:, 0, 2:4, :], in_=eta2[:, 1, :])
            nc.gpsimd.dma_start(out=penr_sb[:, 0:GW], in_=penr[:, 0:GW])
            nc.gpsimd.dma_start(out=et6[:, 1, 0:2, :], in_=etb2[:, 0, :])
            nc.gpsimd.dma_start(out=et6[:, 1, 2:4, :], in_=etb2[:, 1, :])
            nc.gpsimd.dma_start(out=penr_sb[:, GW:B], in_=penr[:, GW:B])

            # --- PE warmup: dense matmuls ramp the p-state while DMAs run --
            warm = etp.tile([P, NW], fp16, tag="warm")
            nc.vector.memset(warm, 0.0)
            wps = psp.tile([P, GW], f32, tag="ps", name="wps")
            for _ in range(8):
                nc.tensor.matmul(wps[:, 0:NW], warm[:, 0:P], warm,
                                 start=True, stop=True)
            ident = etp.tile([P, P], f32, tag="ident")
            from concourse.masks import make_identity
            make_identity(nc, ident)

            for r in range(reps):
                out_sb = accp.tile([P, 2 * MT], f32, tag="osb", name="osb")
                hx_accs = [accp.tile([P, NG], f32, tag="hx", name=f"hx{t}")
                           for t in range(MT)]
                # one scratch slice per (g,t) tile; last columns are combined
                # in a single batched strided TT at the end
                scr8 = wp.tile([P, NG * MT * GW], f32, tag="scr", name="scr8")
                scr3 = scr8.rearrange("p (i n) -> p i n", n=GW)
                for g in range(NG):
                  for t in range(MT):
                    ms = slice(t * P, (t + 1) * P)
                    hx_acc = hx_accs[t]
                    last = g == NG - 1
                    ps = psp.tile([P, GW], f32, tag="ps", name="ps")
                    # gram DR matmuls ordered by k-pair so each DMA chunk
                    # unblocks a full wave; pen (stop=True) last
                    for dr in range(NDR):
                        for j in range(GW // NW):
                            js = slice(j * NW, (j + 1) * NW)
                            nc.tensor.matmul(
                                ps[:, js],
                                et6[:, 0, 2 * dr:2 * dr + 2, ms],
                                et6[:, g, 2 * dr:2 * dr + 2, js],
                                start=(dr == 0), stop=False,
                                perf_mode=mybir.MatmulPerfMode.DoubleRow,
                            )
                    for j in range(GW // NW):
                        js = slice(j * NW, (j + 1) * NW)
                        cs = slice(g * GW + j * NW, g * GW + (j + 1) * NW)
                        nc.tensor.matmul(
                            ps[:, js],
                            penl_sb[:, ms], penr_sb[:, cs],
                            start=False, stop=True,
                        )
                    # fused mining: accum_out = row-max over ALL cols
                    # (chained via s1), scratch[:, -1] = row-min (scan).
                    # No PSUM-reading fixups: PSUM frees at mining end.
                    i8 = g * MT + t
                    nc.vector._custom_dve(
                        DUAL_OP,
                        out=scr8[:, i8 * GW:(i8 + 1) * GW],
                        in0=ps,
                        s0=float(GW - 1),
                        s1=(NEG_INIT if g == 0 else hx_acc[:, g - 1:g]),
                        imm2=SCAN_INIT,
                        accum_out=(out_sb[:, t:t + 1] if last
                                   else hx_acc[:, g:g + 1]),
                    )
                # min side: combine the 8 scan results in one strided TT
                nc.vector.tensor_tensor(
                    out_sb[:, MT:2 * MT],
                    scr3[:, 0:MT, GW - 1:GW],
                    scr3[:, MT:2 * MT, GW - 1:GW],
                    mybir.AluOpType.min,
                )
                # pack outputs: [128, 8] -> [8, 128] via PE transpose, one DMA
                tr = psp.tile([P, GW], f32, tag="ps", name="tr")
                nc.tensor.transpose(tr[0:2 * MT, 0:P], out_sb, ident)
                out_tr = accp.tile([P, P], f32, tag="otr", name="otr")
                nc.vector.tensor_copy(out_tr[0:2 * MT, :], tr[0:2 * MT, 0:P])
                nc.sync.dma_start(out=outd[r], in_=out_tr[0:2 * MT, :])
    nc.compile()
    return nc


def _get_nc(reps=1, kp=KP):
    if (reps, kp) not in _nc_cache:
        _nc_cache[(reps, kp)] = _build(reps, kp)
    return _nc_cache[(reps, kp)]


FP8 = ml_dtypes.float8_e4m3


def _prepare_inputs(embeddings, labels):
    Ef = np.ascontiguousarray(np.asarray(embeddings, dtype=np.float32))
    lab = np.asarray(labels).astype(np.int64)
    perm = np.argsort(lab, kind="stable")
    Ef = Ef[perm]
    labp = lab[perm]

    # fp8 embeddings, scaled so the gram gives 2 e.e directly
    et8 = np.ascontiguousarray(
        (Ef * np.float32(np.sqrt(2.0))).T.astype(FP8))        # [D, B]
    et8f = et8.astype(np.float32)
    # [P, KT, B]: partition-major view for contiguous DMA rows
    et8p = np.ascontiguousarray(et8.reshape(KT, P, B).transpose(1, 0, 2))

    s = Ef.sum(axis=1, dtype=np.float32)
    sq_q = (et8f * et8f).sum(axis=0) / np.float32(2.0)        # |e_q|^2
    colterm = (sq_q - 2.0 * EPS * s).astype(np.float32)
    rowterm = (sq_q + 2.0 * EPS * s + D * EPS * EPS).astype(np.float32)

    # colterm split into three fp8 rows against lhs = -4 (e4m3 max is 240)
    c4 = colterm / np.float32(4.0)
    h = c4.astype(FP8)
    r1 = c4 - h.astype(np.float32)
    l1 = r1.astype(FP8)
    l2 = (r1 - l1.astype(np.float32)).astype(FP8)

    pen_l = np.float32(128.0)   # lhs indicator value; lhs*rhs = -PEN
    pen_r = np.float32(-128.0)

    ncls_max = max(
        len(np.unique(labp[c * ROWS:(c + 1) * ROWS])) for c in range(N_CORES))
    kp = KP if ncls_max <= KP - 3 else KP_BIG

    in_maps = []
    for c in range(N_CORES):
        r0, r1_ = c * ROWS, (c + 1) * ROWS
        # column permutation: [own 512 | rest of own group | other group]
        g_own = (c * ROWS) // GW
        own = np.arange(r0, r1_)
        grp = np.arange(g_own * GW, (g_own + 1) * GW)
        rest = grp[~np.isin(grp, own)]
        other = np.arange((1 - g_own) * GW, (2 - g_own) * GW)
        colp = np.concatenate([own, rest, other])

        cls_ids = np.unique(labp[r0:r1_])
        ncls = len(cls_ids)
        assert ncls <= kp - 2, ncls
        use_lolo = ncls <= kp - 3
        dim_of = {q: i for i, q in enumerate(cls_ids)}

        penl_a = np.zeros((kp, ROWS), dtype=FP8)
        for i in range(ROWS):
            penl_a[dim_of[labp[r0 + i]], i] = pen_l
        penl_a[kp - 2, :] = FP8(-4.0)
        penl_a[kp - 1, :] = FP8(-4.0)
        if use_lolo:
            penl_a[kp - 3, :] = FP8(-4.0)

        penr_a = np.zeros((kp, B), dtype=FP8)
        labc = labp[colp]
        for q in cls_ids:
            penr_a[dim_of[q], labc == q] = pen_r
        penr_a[kp - 2, :] = h[colp]
        penr_a[kp - 1, :] = l1[colp]
        if use_lolo:
            penr_a[kp - 3, :] = l2[colp]

        in_maps.append({
            "eta": np.ascontiguousarray(
                et8p[:, :, colp[:GW]]).reshape(P, KT * GW),
            "etb": np.ascontiguousarray(
                et8p[:, :, colp[GW:]]).reshape(P, KT * GW),
            "penl": penl_a,
            "penr": penr_a,
        })
    return in_maps, labp, rowterm, kp


def _postprocess(results, labp, rowterm):
    # slot t   = accum (row max of w)  -> hardest negative
    # slot MT+t = scan (row min of w)  -> hardest positive
    mx_raw = np.concatenate([r["out"][0][:MT].reshape(-1) for r in results])
    mn_raw = np.concatenate([r["out"][0][MT:].reshape(-1) for r in results])
    hn2 = rowterm - mx_raw
    hp2 = rowterm - np.float32(PEN) - mn_raw
    hp = np.sqrt(np.maximum(hp2, 0.0, dtype=np.float32))
    hn = np.sqrt(np.maximum(hn2, 0.0, dtype=np.float32))

    cnt_lab = np.bincount(labp, minlength=1)
    n_same = cnt_lab[labp]
    valid = (n_same > 1) & (n_same < B)
    per = np.where(valid, np.maximum(hp - hn + np.float32(MARGIN), 0.0), 0.0)
    cnt = np.float32(valid.sum())
    if cnt > 0:
        loss = np.float32(per.sum(dtype=np.float32) / max(cnt, np.float32(1.0)))
    else:
        loss = np.float32(0.0)
    return np.asarray(loss, dtype=np.float32)


def _run(in_maps, reps=1, kp=KP, **kw):
    nc = _get_nc(reps, kp)
    return run_bass_kernel_spmd(nc, in_maps, core_ids=list(range(N_CORES)), **kw)


def kernel(embeddings, labels):
    in_maps, labp, rowterm, kp = _prepare_inputs(embeddings, labels)
    res = _run(in_maps, kp=kp)
    return _postprocess(res.results, labp, rowterm)


# revision 22
# speedup vs baseline: 1.0324x; 1.0324x over previous
"""BatchHardTripletLoss on 8 Trainium2 NeuronCores — fp8 version.

Strategy (data-parallel over anchor rows, samples pre-sorted by label):
  - host sorts samples by label; core c owns anchor rows [c*512, (c+1)*512).
  - mining domain is flipped so the embedding matrix is used unsigned on
    both matmul sides (no separate negated copy):
        w = 2 e_m.e_n - colterm_n - PEN*same(m,n)
    where colterm_n = |e_n|^2 - 2*eps*sum(e_n).  Then per anchor row m:
        row max of w  -> hn2 = rowterm_m - max   (hardest negative^2)
        row min of w  -> hp2 = rowterm_m - PEN - min  (hardest positive^2)
    PEN pushes same-label columns below every non-same value, so the
    global max is the hardest negative and the global min lands on the
    hardest positive (self-column has d2=0 and never wins generically;
    rows without positives are masked on the host via label counts).
  - gram matmuls run in fp8e4 DoubleRow (2 cols/cycle), contracting 256
    dims per instruction; embeddings are scaled by sqrt(2) so the gram
    directly yields 2 e.e.  colterm/PEN are injected by one K=128 fp8
    matmul per psum slice (class-indicator rows + colterm/4 rows).
  - per-core column permutation [own 512 | rest of own group | other
    group] makes the one SPMD program core-agnostic: the stationary
    (lhsT) slices always sit at columns 0:512 of the first group.
  - DVE mines row max+min in one custom pass per [128, 2048] psum tile.
  - host: validity via label bincount, sqrt, mean.
"""

import dataclasses

import numpy as np
import ml_dtypes

import concourse.bacc as bacc
import concourse.mybir as mybir
from concourse.bass_utils import run_bass_kernel_spmd
from concourse.tile import TileContext
from concourse import dve_ops as _dve_ops
from concourse.dve_spec import (
    AluOp, C0, C1, C2, Idx, Spec, Src0, lower, ne, scan, select,
)
from concourse.dve_uop import AluInp, DveOpSpec, InpSel


def _register_dual_op():
    """One DVE pass over a [P, N] tile producing BOTH reductions:
    accum_out = max(seed_s1, in0) over ALL elements,
    out[:, N-1] = running min (seeded +FLT_MAX via imm2) = total row min.

    lower() produces an accum that reduces the *body* (which at index s0
    carries the scan value instead of the element), requiring a one-column
    fixup that reads PSUM and stalls buffer release.  We patch the steady
    uop's accum stage to read the Src0 delay lane directly, so the max
    covers every element and no fixup is needed.
    """
    name = "ANT_MINMAX_DUAL2"
    for op in _dve_ops.OPS:
        if op.name == name:
            return op
    spec = Spec(
        body=select(ne(Idx, C0), Src0, scan(AluOp.MIN, Src0, init=C2)),
        accum=AluOp.MAX,
        accum_init=C1,
        reference=lambda in0, s0, s1, imm2: np.where(
            np.arange(in0.shape[-1]) != s0,
            in0,
            np.minimum.accumulate(np.minimum(in0, imm2), axis=-1),
        ),
    )
    op = _dve_ops.DveOp(name, spec, subdim=False, uops_sha={})
    _dve_ops.OPS.append(op)
    opcode = _dve_ops._CUSTOM_DVE_ROW_BASE + len(_dve_ops.OPS) - 1
    assert opcode < 0x20
    _dve_ops._SUB_OPCODE_FOR_NAME[name] = opcode
    _dve_ops.CUSTOM_DVE_SPECS[name] = spec
    shas = {}
    for ver in ("v3", "v4"):
        uops = lower(spec, ver=ver)
        steady = uops[-1]
        src0_lane = next(
            i - 1 for i, (sel, en) in enumerate(zip(steady.inp,
                                                    steady.inp_enable))
            if en and sel == InpSel.SRC_0)
        patched = False
        for dp in steady.datapath_config:
            if (dp.op == AluOp.MAX and dp.alu_out_a_enable
                    and dp.alu_src0 == AluInp.CURR_ALU_OUT):
                dp.alu_src1 = AluInp(int(AluInp.PREV_DELAY_0) + src0_lane)
                patched = True
                break
        assert patched, f"accum stage not found in {ver} steady uop"
        s = DveOpSpec(name=name, opcode=opcode, uops=uops, rd1_en=False)
        shas[ver] = s.sha(ver)
        _dve_ops._COMPILE_CACHE[(name, ver)] = s
    op = dataclasses.replace(op, uops_sha=shas)
    _dve_ops.OPS[-1] = op
    return op


DUAL_OP = _register_dual_op()

B = 4096          # batch (anchors)
D = 512           # embedding dim
N_CORES = 8
ROWS = B // N_CORES      # 512 anchor rows per core
P = 128                  # partitions
MT = ROWS // P           # 4 m-tiles per core
NW = 512                 # psum bank width (fp32)
GW = 2048                # column group width (4 banks)
NG = B // GW             # 2 column groups
KT = D // P              # 4 contraction tiles
NDR = KT // 2            # 2 DoubleRow matmuls cover K=512

PEN = 16384.0            # same-label penalty; must exceed max d2 (~2.7k here)
KP = 80                  # penalty-matmul contraction (class dims + colterms)
KP_BIG = 128             # fallback when a core has > KP-3 distinct classes
MARGIN = 0.5
EPS = 1e-6

_nc_cache = {}


def _build(reps=1, kp=KP):
    nc = bacc.Bacc("TRN2", target_bir_lowering=False)
    fp8 = mybir.dt.float8e4
    fp16 = mybir.dt.float16
    f32 = mybir.dt.float32

    # [P, KT*GW] partition-major so DMA descriptors are 4KB contiguous runs
    eta = nc.dram_tensor("eta", [P, KT * GW], fp8, kind="ExternalInput")
    etb = nc.dram_tensor("etb", [P, KT * GW], fp8, kind="ExternalInput")
    penl = nc.dram_tensor("penl", [kp, ROWS], fp8, kind="ExternalInput")
    penr = nc.dram_tensor("penr", [kp, B], fp8, kind="ExternalInput")
    outd = nc.dram_tensor("out", [reps, 2 * MT, P], f32, kind="ExternalOutput")

    NEG_INIT = -3.0e38
    SCAN_INIT = 3.0e38
    with TileContext(nc) as tc:
        with (
            tc.tile_pool(name="etp", bufs=1) as etp,
            tc.tile_pool(name="wp", bufs=1) as wp,
            tc.tile_pool(name="accp", bufs=MT) as accp,
            tc.tile_pool(name="psp", bufs=2, space="PSUM") as psp,
        ):
            # --- input DMAs first, split by (group, k-pair) chunks that are
            # contiguous 4KB-per-partition runs on BOTH sides; first chunk on
            # the sync queue so both DGEs issue in parallel.
            et_all = etp.tile([P, NG * KT * GW], fp8, tag="et", name="et_all")
            et6 = et_all.rearrange("p (g k n) -> p g k n", g=NG, k=KT)
            eta2 = eta.rearrange("p (c n) -> p c n", c=2)   # k-pair chunks
            etb2 = etb.rearrange("p (c n) -> p c n", c=2)
            penl_sb = etp.tile([kp, ROWS], fp8, tag="penl")
            penr_sb = etp.tile([kp, B], fp8, tag="penr")
            nc.sync.dma_start(out=et6[:, 0, 0:2, :], in_=eta2[:, 0, :])
            nc.sync.dma_start(out=penr_sb[:, 0:GW], in_=penr[:, 0:GW])
            nc.gpsimd.dma_start(out=penl_sb, in_=penl[:, :])
            nc.gpsimd.dma_start(out=et6[:, 0, 2:4, :], in_=eta2[:, 1, :])
            nc.gpsimd.dma_start(out=et6[:, 1, 0:2, :], in_=etb2[:, 0, :])
            nc.gpsimd.dma_start(out=et6[:, 1, 2:4, :], in_=etb2[:, 1, :])
            nc.gpsimd.dma_start(out=penr_sb[:, GW:B], in_=penr[:, GW:B])

            # --- PE warmup: dense matmuls ramp the p-state while DMAs run --
            warm = etp.tile([P, NW], fp16, tag="warm")
            nc.vector.memset(warm, 0.0)
            wps = psp.tile([P, GW], f32, tag="ps", name="wps")
            for _ in range(5):
                nc.tensor.matmul(wps[:, 0:NW], warm[:, 0:P], warm,
                                 start=True, stop=True)
            ident = etp.tile([P, P], f32, tag="ident")
            from concourse.masks import make_identity
            make_identity(nc, ident)

            for r in range(reps):
                out_sb = accp.tile([P, 2 * MT], f32, tag="osb", name="osb")
                hx_accs = [accp.tile([P, NG], f32, tag="hx", name=f"hx{t}")
                           for t in range(MT)]
                # one scratch slice per (g,t) tile; last columns are combined
                # in a single batched strided TT at the end
                scr8 = wp.tile([P, NG * MT * GW], f32, tag="scr", name="scr8")
                scr3 = scr8.rearrange("p (i n) -> p i n", n=GW)
                for g in range(NG):
                  for t in range(MT):
                    ms = slice(t * P, (t + 1) * P)
                    hx_acc = hx_accs[t]
                    last = g == NG - 1
                    ps = psp.tile([P, GW], f32, tag="ps", name="ps")
                    # gram DR matmuls ordered by k-pair so each DMA chunk
                    # unblocks a full wave; pen (stop=True) last
                    for dr in range(NDR):
                        for j in range(GW // NW):
                            js = slice(j * NW, (j + 1) * NW)
                            nc.tensor.matmul(
                                ps[:, js],
                                et6[:, 0, 2 * dr:2 * dr + 2, ms],
                                et6[:, g, 2 * dr:2 * dr + 2, js],
                                start=(dr == 0), stop=False,
                                perf_mode=mybir.MatmulPerfMode.DoubleRow,
                            )
                    for j in range(GW // NW):
                        js = slice(j * NW, (j + 1) * NW)
                        cs = slice(g * GW + j * NW, g * GW + (j + 1) * NW)
                        nc.tensor.matmul(
                            ps[:, js],
                            penl_sb[:, ms], penr_sb[:, cs],
                            start=False, stop=True,
                        )
                    # fused mining: accum_out = row-max over ALL cols
                    # (chained via s1), scratch[:, -1] = row-min (scan).
                    # No PSUM-reading fixups: PSUM frees at mining end.
                    i8 = g * MT + t
                    nc.vector._custom_dve(
                        DUAL_OP,
                        out=scr8[:, i8 * GW:(i8 + 1) * GW],
                        in0=ps,
                        s0=float(GW - 1),
                        s1=(NEG_INIT if g == 0 else hx_acc[:, g - 1:g]),
                        imm2=SCAN_INIT,
                        accum_out=(out_sb[:, t:t + 1] if last
                                   else hx_acc[:, g:g + 1]),
                    )
                # min side: combine the 8 scan results in one strided TT
                nc.vector.tensor_tensor(
                    out_sb[:, MT:2 * MT],
                    scr3[:, 0:MT, GW - 1:GW],
                    scr3[:, MT:2 * MT, GW - 1:GW],
                    mybir.AluOpType.min,
                )
                # pack outputs: [128, 8] -> [8, 128] via PE transpose, one DMA
                tr = psp.tile([P, GW], f32, tag="ps", name="tr")
                nc.tensor.transpose(tr[0:2 * MT, 0:P], out_sb, ident)
                out_tr = accp.tile([P, P], f32, tag="otr", name="otr")
                nc.vector.tensor_copy(out_tr[0:2 * MT, :], tr[0:2 * MT, 0:P])
                nc.sync.dma_start(out=outd[r], in_=out_tr[0:2 * MT, :])
    nc.compile()
    return nc


def _get_nc(reps=1, kp=KP):
    if (reps, kp) not in _nc_cache:
        _nc_cache[(reps, kp)] = _build(reps, kp)
    return _nc_cache[(reps, kp)]


FP8 = ml_dtypes.float8_e4m3


def _prepare_inputs(embeddings, labels):
    Ef = np.ascontiguousarray(np.asarray(embeddings, dtype=np.float32))
    lab = np.asarray(labels).astype(np.int64)
    perm = np.argsort(lab, kind="stable")
    Ef = Ef[perm]
    labp = lab[perm]

    # fp8 embeddings, scaled so the gram gives 2 e.e directly
    et8 = np.ascontiguousarray(
        (Ef * np.float32(np.sqrt(2.0))).T.astype(FP8))        # [D, B]
    et8f = et8.astype(np.float32)
    # [P, KT, B]: partition-major view for contiguous DMA rows
    et8p = np.ascontiguousarray(et8.reshape(KT, P, B).transpose(1, 0, 2))

    s = Ef.sum(axis=1, dtype=np.float32)
    sq_q = (et8f * et8f).sum(axis=0) / np.float32(2.0)        # |e_q|^2
    colterm = (sq_q - 2.0 * EPS * s).astype(np.float32)
    rowterm = (sq_q + 2.0 * EPS * s + D * EPS * EPS).astype(np.float32)

    # colterm split into three fp8 rows against lhs = -4 (e4m3 max is 240)
    c4 = colterm / np.float32(4.0)
    h = c4.astype(FP8)
    r1 = c4 - h.astype(np.float32)
    l1 = r1.astype(FP8)
    l2 = (r1 - l1.astype(np.float32)).astype(FP8)

    pen_l = np.float32(128.0)   # lhs indicator value; lhs*rhs = -PEN
    pen_r = np.float32(-128.0)

    ncls_max = max(
        len(np.unique(labp[c * ROWS:(c + 1) * ROWS])) for c in range(N_CORES))
    kp = KP if ncls_max <= KP - 3 else KP_BIG

    in_maps = []
    for c in range(N_CORES):
        r0, r1_ = c * ROWS, (c + 1) * ROWS
        # column permutation: [own 512 | rest of own group | other group]
        g_own = (c * ROWS) // GW
        own = np.arange(r0, r1_)
        grp = np.arange(g_own * GW, (g_own + 1) * GW)
        rest = grp[~np.isin(grp, own)]
        other = np.arange((1 - g_own) * GW, (2 - g_own) * GW)
        colp = np.concatenate([own, rest, other])

        cls_ids = np.unique(labp[r0:r1_])
        ncls = len(cls_ids)
        assert ncls <= kp - 2, ncls
        use_lolo = ncls <= kp - 3
        dim_of = {q: i for i, q in enumerate(cls_ids)}

        penl_a = np.zeros((kp, ROWS), dtype=FP8)
        for i in range(ROWS):
            penl_a[dim_of[labp[r0 + i]], i] = pen_l
        penl_a[kp - 2, :] = FP8(-4.0)
        penl_a[kp - 1, :] = FP8(-4.0)
        if use_lolo:
            penl_a[kp - 3, :] = FP8(-4.0)

        penr_a = np.zeros((kp, B), dtype=FP8)
        labc = labp[colp]
        for q in cls_ids:
            penr_a[dim_of[q], labc == q] = pen_r
        penr_a[kp - 2, :] = h[colp]
        penr_a[kp - 1, :] = l1[colp]
        if use_lolo:
            penr_a[kp - 3, :] = l2[colp]

        in_maps.append({
            "eta": np.ascontiguousarray(
                et8p[:, :, colp[:GW]]).reshape(P, KT * GW),
            "etb": np.ascontiguousarray(
                et8p[:, :, colp[GW:]]).reshape(P, KT * GW),
            "penl": penl_a,
            "penr": penr_a,
        })
    return in_maps, labp, rowterm, kp


def _postprocess(results, labp, rowterm):
    # slot t   = accum (row max of w)  -> hardest negative
    # slot MT+t = scan (row min of w)  -> hardest positive
    mx_raw = np.concatenate([r["out"][0][:MT].reshape(-1) for r in results])
    mn_raw = np.concatenate([r["out"][0][MT:].reshape(-1) for r in results])
    hn2 = rowterm - mx_raw
    hp2 = rowterm - np.float32(PEN) - mn_raw
    hp = np.sqrt(np.maximum(hp2, 0.0, dtype=np.float32))
    hn = np.sqrt(np.maximum(hn2, 0.0, dtype=np.float32))

    cnt_lab = np.bincount(labp, minlength=1)
    n_same = cnt_lab[labp]
    valid = (n_same > 1) & (n_same < B)
    per = np.where(valid, np.maximum(hp - hn + np.float32(MARGIN), 0.0), 0.0)
    cnt = np.float32(valid.sum())
    if cnt > 0:
        loss = np.float32(per.sum(dtype=np.float32) / max(cnt, np.float32(1.0)))
    else:
        loss = np.float32(0.0)
    return np.asarray(loss, dtype=np.float32)


def _run(in_maps, reps=1, kp=KP, **kw):
    nc = _get_nc(reps, kp)
    return run_bass_kernel_spmd(nc, in_maps, core_ids=list(range(N_CORES)), **kw)


def kernel(embeddings, labels):
    in_maps, labp, rowterm, kp = _prepare_inputs(embeddings, labels)
    res = _run(in_maps, kp=kp)
    return _postprocess(res.results, labp, rowterm)
